# revision 12
# baseline (speedup 1.0000x reference)
"""BitConv2d (BitNet-style fake-quant 3x3 conv) Trainium2 Bass kernel.

Reference computation (see problem):
  ws   = max(mean|w|, 1e-6);  qw = clip(round(w/ws), -1, 1)           (per-tensor ternary)
  amax = max(max|x| over (N,H,W) per channel, 1e-6); xs = 127/amax
  qx   = clip(round(x*xs), -128, 127)                                  (per-channel int8)
  out  = conv2d(qx/xs, qw*ws, stride 1, pad 1, NCHW/OIHW) + bias

Key algebraic restructuring for the tensor engine:
  out[n,o,h,w] = sum_{c,i,j} qx[n,c,h+i-1,w+j-1] * (qw[o,c,i,j] * ws * amax[c]/127)
so the conv runs as bf16 matmuls with
  rhs  = qx          (integers in [-127,127]  -> EXACT in bf16)
  lhsT = qw * s_c    (ternary * per-in-channel scale, bf16-rounded once per channel)
accumulated in fp32 PSUM. The 3x3 conv is 18 accumulating matmuls
(2 cin-tiles x 9 taps) over a zero-padded flat spatial layout with row
stride 57 (one left-pad column per row doubles as the previous row's right
pad), where each tap is a constant flat column offset di*57+dj.

Sharding: data-parallel over batch (4 images/core on 8 cores), weight
replicated (ws computed redundantly); per-channel amax needs a global max.

v3 startup pipeline: the global amax AllReduce is SPLIT per cin-half.
x streams ct-major (all images' channels 0..127 first), so CC0 (amax of
ct0) triggers at ~2/3 of the x-load time and CC1 right after the load.
All weight prep (ws, ternarize on gpsimd, PE transposes) happens under
CC0's latency using only local data. When CC0 lands, the conv starts on
ct0-only accumulation groups across 7 PSUM banks (~15us of PE work)
while CC1 is still in flight; each group is closed by its 9 ct1 taps
once CC1's scales arrive. All of x stays resident in SBUF (no reloads);
x owns the SP hardware DMA queue, weights/collective hops ride the
Activation hardware queue (the gpsimd software queue's ~8us completion
latency would sit on the collective critical path).
"""

import sys
import types

for _p in ("/opt/trn_rl_repo", "/root/.axon_site/_ro/trn_rl_repo"):
    if _p not in sys.path:
        sys.path.insert(0, _p)

import numpy as np
import ml_dtypes

import concourse.bacc as bacc
import concourse.mybir as mybir
import concourse.tile as tile
from concourse.bass_utils import run_bass_kernel_spmd
from concourse.tile_rust import add_dep_helper

F32 = mybir.dt.float32
BF16 = mybir.dt.bfloat16
ALU = mybir.AluOpType
AX = mybir.AxisListType
AF = mybir.ActivationFunctionType

N_CORES = 8
N, CIN, H, W = 32, 256, 56, 56
COUT, KH, KW = 256, 3, 3
NPC = N // N_CORES          # images per core
HW = H * W                  # 3136
PW = W + 1                  # 57: padded row stride (left pad doubles as right pad)
QCOLS = 3312                # >= (55+2)*57 + 58 = 3307, 8-aligned
ROWS_PER_CHUNK = 8
CHUNK = ROWS_PER_CHUNK * PW   # 456 psum cols per chunk (<=512, one bank)
NCHUNK = H // ROWS_PER_CHUNK  # 7
OUT_CHUNK = ROWS_PER_CHUNK * W  # 448 valid cols per chunk
MAGIC = 12582912.0          # 1.5*2^23: (v+MAGIC)-MAGIC == round-half-even(v)
EPS = 1e-6
FAN = COUT * CIN * KH * KW  # weight element count for mean|w|
NGRP = NPC * 2 * NCHUNK     # 56 (image, cout-tile, chunk) psum groups
AHEAD = 7                   # ct0-only groups in flight while CC1 is pending


def _build_program():
    nc = bacc.Bacc(
        "TRN2",
        target_bir_lowering=False,
        debug=False,
        enable_asserts=False,
        num_devices=N_CORES,
    )
    x_d = nc.dram_tensor("x", [NPC, CIN, H, W], F32, kind="ExternalInput")
    w_d = nc.dram_tensor("weight", [COUT, CIN, KH, KW], F32, kind="ExternalInput")
    b_d = nc.dram_tensor("bias", [COUT], F32, kind="ExternalInput")
    o_d = nc.dram_tensor("out", [NPC, COUT, H, W], F32, kind="ExternalOutput")
    ident_d = nc.inline_tensor(np.eye(128, dtype=ml_dtypes.bfloat16), name="ident")

    x_flat = x_d.ap().rearrange("n c h w -> n c (h w)")
    o_flat = o_d.ap().rearrange("n c h w -> n c (h w)")
    w_flat = w_d.ap().rearrange("o c kh kw -> o (c kh kw)")  # free idx = c*9 + tap

    with tile.TileContext(nc) as tc:
        with tc.tile_pool(name="persist", bufs=1) as pp, \
             tc.tile_pool(name="dram", bufs=1, space="DRAM") as dram:
            # ---- persistent tiles ----
            qx = [pp.tile([128, QCOLS], BF16, name=f"qx{i}") for i in range(NPC * 2)]
            # 36 weight tiles; idx = ct*18 + ot*9 + tap; scaled in place post-CC
            lhsT = pp.tile([128, 36, 128], BF16, name="lhsT")
            ident_sb = pp.tile([128, 128], BF16, name="ident_sb")
            # all small scalars packed into one tile (slots are 4KB-padded)
            misc = pp.tile([128, 160], F32, name="misc")
            ones_m = misc[0:1, 0:128]
            ones_k = misc[:, 128:129]
            bias_sb = misc[:, 130:132]
            wsb = misc[:, 132:134]     # col0 = ws, col1 = 1/ws
            xs = misc[:, 134:136]      # 127/amax, per ct half
            sc = misc[:, 136:138]      # ws*amax/127, per ct half
            amax2 = misc[:, 138:140]
            # partial amax: ct0 at cols 0..4 (n0,n1,n2,n3-half,n3-half),
            # ct1 at cols 5..9; the last image of each half is reduced in
            # halves so only a half-reduce trails its DMA
            pamax = misc[:, 140:150]
            ws1 = misc[0:1, 150:152]
            absw = misc[:, 152:154]
            negmagic = misc[:, 154:155]
            cc_in = [dram.tile([128, 1], F32, name=f"cc_in{i}") for i in range(2)]
            cc_out = [dram.tile([128, 1], F32, name=f"cc_out{i}",
                                addr_space="Shared") for i in range(2)]

            # ---- weights + constants first on the Act HW queue: the x
            # stream owns the SP queue, and HBM is idle for the first ~10us
            # while the rings spin up, so this is free bandwidth ----
            wt_scope = tc.tile_pool(name="wtmp", bufs=1)
            wp = wt_scope.__enter__()
            wt1 = []
            wdma = []
            for ot in range(2):
                wt = wp.tile([128, CIN * 9], F32, name=f"wt{ot}", tag=f"wt{ot}")
                wdma.append(nc.scalar.dma_start(wt[:],
                                                w_flat[ot * 128:(ot + 1) * 128, :]))
                wt1.append(wt)
            nc.scalar.dma_start(ident_sb[:], ident_d.ap())
            nc.scalar.dma_start(bias_sb, b_d.ap().rearrange("(o p) -> p o", p=128))
            nc.vector.memset(ones_k, 1.0)
            nc.vector.memset(ones_m, 1.0)
            nc.vector.memset(negmagic, -MAGIC)

            # ---- pass A: stream x on the Sync HW queue, ct-major so the
            # ct0 collective can trigger before the ct1 tiles land. Every
            # tile stays resident in SBUF (~98KB/partition): no reloads. ----
            xres = {}
            for ct in range(2):
                for n in range(NPC):
                    t = pp.tile([128, HW], F32, name=f"xa{n}_{ct}")
                    src = x_flat[n, ct * 128:(ct + 1) * 128, :]
                    if n == NPC - 1:
                        # split the last tile of the half so only a
                        # half-reduce trails the collective trigger
                        nc.sync.dma_start(t[:, 0:HW // 2], src[:, 0:HW // 2])
                        nc.vector.reduce_max(pamax[:, 5 * ct + 3:5 * ct + 4],
                                             t[:, 0:HW // 2], axis=AX.X,
                                             apply_absolute_value=True)
                        nc.sync.dma_start(t[:, HW // 2:], src[:, HW // 2:])
                        nc.vector.reduce_max(pamax[:, 5 * ct + 4:5 * ct + 5],
                                             t[:, HW // 2:], axis=AX.X,
                                             apply_absolute_value=True)
                    else:
                        nc.sync.dma_start(t[:], src)
                        nc.vector.reduce_max(pamax[:, 5 * ct + n:5 * ct + n + 1],
                                             t[:], axis=AX.X,
                                             apply_absolute_value=True)
                    xres[(n, ct)] = t
                if ct == 0:
                    am0 = nc.vector.reduce_max(amax2[:, 0:1], pamax[:, 0:5],
                                               axis=AX.X)
                else:
                    am1 = nc.vector.reduce_max(amax2[:, 1:2], pamax[:, 5:10],
                                               axis=AX.X)

            # ---- qx zero-fill: only the padding cells are ever read and
            # never overwritten, so memset just those (head row, the
            # per-row wrap column, tail) on gpsimd ----
            for i in range(NPC * 2):
                nc.gpsimd.memset(qx[i][:, 0:PW + 1], 0.0)
                nc.gpsimd.memset(
                    qx[i][:, PW * 2:PW * 2 + H * PW].rearrange(
                        "p (h w) -> p h w", w=PW)[:, :, 0:1],
                    0.0,
                )
                nc.gpsimd.memset(qx[i][:, PW * 2 + H * PW - PW:QCOLS], 0.0)

            with tc.tile_pool(name="psum_t", bufs=4, space="PSUM") as pt_pool, \
                 tc.tile_pool(name="psum_s", bufs=1, space="PSUM") as ps_pool:
                # ---- ws = max(mean|w|,eps): per-partition |w|-sums on
                # vector (gpsimd elementwise measured ~17x slower -- never
                # offload bulk math there), total via PE with ones ----
                for ot in range(2):
                    ar = nc.vector.reduce_sum(absw[:, ot:ot + 1], wt1[ot][:],
                                              axis=AX.X,
                                              apply_absolute_value=True)
                    if ot == 0:
                        add_dep_helper(ar.ins, am1.ins,
                                       reason="absw after amax reduces")
                nc.vector.tensor_add(absw[:, 0:1], absw[:, 0:1], absw[:, 1:2])
                ps_s = ps_pool.tile([1, 1], F32, name="ps_s")
                nc.tensor.matmul(ps_s[:], ones_k, absw[:, 0:1], start=True,
                                 stop=True)
                nc.vector.tensor_scalar(ws1[:, 0:1], ps_s[:], 1.0 / FAN, EPS,
                                        op0=ALU.mult, op1=ALU.max)
                nc.vector.reciprocal(ws1[:, 1:2], ws1[:, 0:1])
                ps_b = ps_pool.tile([128, 2], F32, name="ps_b")
                nc.tensor.matmul(ps_b[:], ones_m, ws1[:, :], start=True, stop=True)
                wsb_cp = nc.scalar.copy(wsb, ps_b[:])

                # ---- collectives: input hop rides the Act HW queue (fast
                # completion), trigger from gpsimd (required engine). The
                # ct0 collective fires ~13us before the full amax would be
                # ready; CC1 follows as soon as the x stream drains. ----
                d0 = nc.scalar.dma_start(cc_in[0][:], amax2[:, 0:1])
                add_dep_helper(d0.ins, wsb_cp.ins,
                               reason="keep weight-prep ACT ops ahead of cc hop")
                cc0 = nc.gpsimd.collective_compute(
                    "AllReduce", ALU.max,
                    replica_groups=[list(range(N_CORES))],
                    ins=[cc_in[0].opt()], outs=[cc_out[0].opt()],
                )
                d1 = nc.scalar.dma_start(cc_in[1][:], amax2[:, 1:2])
                cc1 = nc.gpsimd.collective_compute(
                    "AllReduce", ALU.max,
                    replica_groups=[list(range(N_CORES))],
                    ins=[cc_in[1].opt()], outs=[cc_out[1].opt()],
                )
                r0 = nc.scalar.dma_start(amax2[:, 0:1], cc_out[0][:])
                r1 = nc.scalar.dma_start(amax2[:, 1:2], cc_out[1][:])

                # ---- ternary quantize qw = clip(round(w/ws), -1, 1) on
                # vector, ordered AFTER the amax-critical reduces (it's
                # local data, needed no earlier than CC0's return), then
                # PE-transpose each 128x128 block ----
                for ot in range(2):
                    wt = wt1[ot]
                    q1 = nc.vector.tensor_scalar(wt[:], wt[:], wsb[:, 1:2], MAGIC,
                                                 op0=ALU.mult, op1=ALU.add)
                    if ot == 0:
                        add_dep_helper(q1.ins, am1.ins,
                                       reason="ternarize after amax reduces")
                    nc.vector.tensor_scalar_sub(wt[:], wt[:], MAGIC)
                    qwb = wp.tile([128, CIN * 9], BF16, name="qwb", tag="qwb",
                                  bufs=2)
                    nc.vector.tensor_scalar(qwb[:], wt[:], -1.0, 1.0,
                                            op0=ALU.max, op1=ALU.min)
                    wv = qwb.rearrange("p (c t) -> p t c", t=9)
                    for ct in range(2):
                        for tap in range(9):
                            idx = ct * 18 + ot * 9 + tap
                            pt = pt_pool.tile([128, 128], BF16, name="pt", tag="pt")
                            nc.tensor.transpose(
                                pt[:],
                                wv[:, tap, ct * 128:(ct + 1) * 128],
                                ident_sb[:],
                            )
                            nc.scalar.copy(lhsT[:, idx, :], pt[:])
            wt_scope.__exit__(None, None, None)

            # ---- post-CC0: scales for the ct0 half; fold s_c into lhsT ----
            x0 = nc.vector.tensor_scalar_max(amax2[:, 0:1], amax2[:, 0:1], EPS)
            # anti-hoist: the in-order vector queue must finish the amax
            # reduces before parking on the CC0-gated ops
            add_dep_helper(x0.ins, am1.ins, reason="post-cc0 after amax reduces")
            nc.vector.reciprocal(xs[:, 0:1], amax2[:, 0:1])
            nc.vector.tensor_scalar_mul(xs[:, 0:1], xs[:, 0:1], 127.0)
            nc.vector.tensor_scalar(sc[:, 0:1], amax2[:, 0:1], wsb[:, 0:1],
                                    1.0 / 127.0, op0=ALU.mult, op1=ALU.mult)

            def scale_lhsT(ct, ot):
                nc.vector.tensor_scalar_mul(
                    lhsT[:, ct * 18 + ot * 9:ct * 18 + (ot + 1) * 9, :],
                    lhsT[:, ct * 18 + ot * 9:ct * 18 + (ot + 1) * 9, :],
                    sc[:, ct:ct + 1],
                )

            def quantize(n, ct, quarters=False):
                # qx = round(x*xs): vector does x*xs+MAGIC in place (fp32),
                # ACT writes qx = t - MAGIC (exact, integer-valued bf16)
                t = xres[(n, ct)]
                tv = t.rearrange("p (h w) -> p h w", w=W)
                qxa = qx[n * 2 + ct][:, PW + 1:PW + 1 + H * PW].rearrange(
                    "p (h w) -> p h w", w=PW)[:, :, 0:W]
                nh = 4 if quarters else 1
                rh = H // nh
                vop = aop = None
                for hh in range(nh):
                    rs = slice(hh * rh, (hh + 1) * rh)
                    vop = nc.vector.tensor_scalar(
                        tv[:, rs, :], tv[:, rs, :],
                        xs[:, ct:ct + 1], MAGIC,
                        op0=ALU.mult, op1=ALU.add)
                    aop = nc.scalar.activation(
                        qxa[:, rs, :], tv[:, rs, :],
                        AF.Identity, bias=negmagic)
                return vop, aop

            scale_lhsT(0, 0)
            q0v, q0a = quantize(0, 0, quarters=True)
            scale_lhsT(0, 1)
            for n in range(1, NPC):
                quantize(n, 0)
            # the CC1 return hop must not park the in-order ACT queue ahead
            # of the first image's ct0 epilogues (they gate the first MM)
            add_dep_helper(r1.ins, q0a.ins,
                           reason="cc1 return after gating ct0 epilogue")

            # ---- post-CC1 scales (vector parks here while the ct0-ahead
            # conv groups below keep the PE busy) ----
            x1 = nc.vector.tensor_scalar_max(amax2[:, 1:2], amax2[:, 1:2], EPS)
            add_dep_helper(x1.ins, q0v.ins,
                           reason="post-cc1 after gating ct0 quantize")
            nc.vector.reciprocal(xs[:, 1:2], amax2[:, 1:2])
            nc.vector.tensor_scalar_mul(xs[:, 1:2], xs[:, 1:2], 127.0)
            nc.vector.tensor_scalar(sc[:, 1:2], amax2[:, 1:2], wsb[:, 0:1],
                                    1.0 / 127.0, op0=ALU.mult, op1=ALU.mult)
            scale_lhsT(1, 0)
            quantize(0, 1, quarters=True)
            scale_lhsT(1, 1)

            # ---- conv: 56 (image, cout-tile, chunk) groups, one PSUM bank
            # each. A group opens with its 9 ct0 taps and closes with the 9
            # ct1 taps + copy-out. AHEAD groups open on ct0 alone, so the
            # PE streams from CC0-land while CC1 is still in flight. ----
            def group_of(k):
                nk, rem = divmod(k, 2 * NCHUNK)
                ot, c8 = divmod(rem, NCHUNK)
                return nk, ot, c8

            def conv_half(ps, k, ct, start):
                nk, ot, c8 = group_of(k)
                base = c8 * CHUNK
                for tap in range(9):
                    di, dj = tap // 3, tap % 3
                    off = base + di * PW + dj
                    nc.tensor.matmul(
                        ps[:, 0:CHUNK],
                        lhsT[:, ct * 18 + ot * 9 + tap, :],
                        qx[nk * 2 + ct][:, off:off + CHUNK],
                        start=(start and tap == 0),
                        stop=(not start and tap == 8),
                        skip_group_check=True,
                    )

            with tc.tile_pool(name="psum_c", bufs=8, space="PSUM") as pc_pool, \
                 tc.tile_pool(name="outp", bufs=6) as op_pool:
                open_ps = {}
                for j in range(AHEAD):
                    open_ps[j] = pc_pool.tile([128, 512], F32, name="ps", tag="ps")
                    conv_half(open_ps[j], j, 0, True)
                for k in range(NGRP):
                    if k % (2 * NCHUNK) == 0 and k // (2 * NCHUNK) + 1 < NPC:
                        quantize(k // (2 * NCHUNK) + 1, 1)
                    ps = open_ps.pop(k)
                    conv_half(ps, k, 1, False)
                    nk, ot, c8 = group_of(k)
                    ob = op_pool.tile([128, OUT_CHUNK], F32, name="ob", tag="ob")
                    nc.scalar.activation(
                        ob.rearrange("p (h w) -> p h w", w=W),
                        ps[:, 0:CHUNK].rearrange(
                            "p (h w) -> p h w", w=PW)[:, :, 0:W],
                        AF.Identity, bias=bias_sb[:, ot:ot + 1])
                    nc.sync.dma_start(
                        o_flat[nk, ot * 128:(ot + 1) * 128,
                               c8 * OUT_CHUNK:(c8 + 1) * OUT_CHUNK],
                        ob[:],
                    )
                    j = k + AHEAD
                    if j < NGRP:
                        open_ps[j] = pc_pool.tile([128, 512], F32,
                                                  name="ps", tag="ps")
                        conv_half(open_ps[j], j, 0, True)

    nc.compile()
    return nc


_NC_CACHE = None


def _get_program():
    global _NC_CACHE
    if _NC_CACHE is None:
        _NC_CACHE = _build_program()
    return _NC_CACHE


def _install_ntff_hook():
    """Register the axon NTFF profiling hook (the antenv stub lacks it)."""
    try:
        import antenv
        if getattr(antenv, "axon_hooks", None) is not None:
            return
        mod = types.ModuleType("antenv.axon_hooks")
        mod._hook = None
        def set_axon_ntff_profile_hook(h):
            mod._hook = h
        def get_axon_ntff_profile_hook():
            return mod._hook
        mod.set_axon_ntff_profile_hook = set_axon_ntff_profile_hook
        mod.get_axon_ntff_profile_hook = get_axon_ntff_profile_hook
        sys.modules["antenv.axon_hooks"] = mod
        antenv.axon_hooks = mod
        from trn_agent_boot.trn_boot import _ntff_profile_via_ctypes
        set_axon_ntff_profile_hook(_ntff_profile_via_ctypes("/opt/axon/libaxon_pjrt.so"))
    except Exception:
        pass


def run(x, weight, bias, trace=False, trace_cores=None):
    x = np.ascontiguousarray(np.asarray(x, dtype=np.float32))
    weight = np.ascontiguousarray(np.asarray(weight, dtype=np.float32))
    bias = np.ascontiguousarray(np.asarray(bias, dtype=np.float32))
    assert x.shape == (N, CIN, H, W), x.shape
    nc = _get_program()
    in_maps = [
        {"x": x[c * NPC:(c + 1) * NPC], "weight": weight, "bias": bias}
        for c in range(N_CORES)
    ]
    if trace:
        _install_ntff_hook()
    res = run_bass_kernel_spmd(nc, in_maps, list(range(N_CORES)), trace=trace,
                               trace_cores=trace_cores)
    out = np.concatenate([res.results[c]["out"] for c in range(N_CORES)], axis=0)
    return out, res


def kernel(x, weight, bias):
    out, _ = run(x, weight, bias, trace=False)
    return out


# revision 16
# speedup vs baseline: 1.0465x; 1.0465x over previous
"""BitConv2d (BitNet-style fake-quant 3x3 conv) Trainium2 Bass kernel.

Reference computation (see problem):
  ws   = max(mean|w|, 1e-6);  qw = clip(round(w/ws), -1, 1)           (per-tensor ternary)
  amax = max(max|x| over (N,H,W) per channel, 1e-6); xs = 127/amax
  qx   = clip(round(x*xs), -128, 127)                                  (per-channel int8)
  out  = conv2d(qx/xs, qw*ws, stride 1, pad 1, NCHW/OIHW) + bias

Key algebraic restructuring for the tensor engine:
  out[n,o,h,w] = sum_{c,i,j} qx[n,c,h+i-1,w+j-1] * (qw[o,c,i,j] * ws * amax[c]/127)
so the conv runs as bf16 matmuls with
  rhs  = qx          (integers in [-127,127]  -> EXACT in bf16)
  lhsT = qw * s_c    (ternary * per-in-channel scale, bf16-rounded once per channel)
accumulated in fp32 PSUM. The 3x3 conv is 18 accumulating matmuls
(2 cin-tiles x 9 taps) over a zero-padded flat spatial layout with row
stride 57 (one left-pad column per row doubles as the previous row's right
pad), where each tap is a constant flat column offset di*57+dj.

Sharding: data-parallel over batch (4 images/core on 8 cores), weight
replicated (ws computed redundantly); per-channel amax needs a global max.

v3 startup pipeline: the global amax AllReduce is SPLIT per cin-half.
x streams ct-major (all images' channels 0..127 first), so CC0 (amax of
ct0) triggers at ~2/3 of the x-load time and CC1 right after the load.
All weight prep (ws, ternarize on gpsimd, PE transposes) happens under
CC0's latency using only local data. When CC0 lands, the conv starts on
ct0-only accumulation groups across 7 PSUM banks (~15us of PE work)
while CC1 is still in flight; each group is closed by its 9 ct1 taps
once CC1's scales arrive. All of x stays resident in SBUF (no reloads);
x owns the SP hardware DMA queue, weights/collective hops ride the
Activation hardware queue (the gpsimd software queue's ~8us completion
latency would sit on the collective critical path).
"""

import sys
import types

for _p in ("/opt/trn_rl_repo", "/root/.axon_site/_ro/trn_rl_repo"):
    if _p not in sys.path:
        sys.path.insert(0, _p)

import numpy as np
import ml_dtypes

import concourse.bacc as bacc
import concourse.mybir as mybir
import concourse.tile as tile
from concourse.bass_utils import run_bass_kernel_spmd
from concourse.tile_rust import add_dep_helper

F32 = mybir.dt.float32
BF16 = mybir.dt.bfloat16
ALU = mybir.AluOpType
AX = mybir.AxisListType
AF = mybir.ActivationFunctionType

N_CORES = 8
N, CIN, H, W = 32, 256, 56, 56
COUT, KH, KW = 256, 3, 3
NPC = N // N_CORES          # images per core
HW = H * W                  # 3136
PW = W + 1                  # 57: padded row stride (left pad doubles as right pad)
QCOLS = 3312                # >= (55+2)*57 + 58 = 3307, 8-aligned
ROWS_PER_CHUNK = 8
CHUNK = ROWS_PER_CHUNK * PW   # 456 psum cols per chunk (<=512, one bank)
NCHUNK = H // ROWS_PER_CHUNK  # 7
OUT_CHUNK = ROWS_PER_CHUNK * W  # 448 valid cols per chunk
MAGIC = 12582912.0          # 1.5*2^23: (v+MAGIC)-MAGIC == round-half-even(v)
EPS = 1e-6
FAN = COUT * CIN * KH * KW  # weight element count for mean|w|
NGRP = NPC * 2 * NCHUNK     # 56 (image, cout-tile, chunk) psum groups
AHEAD = 7                   # ct0-only groups in flight while CC1 is pending


def _build_program():
    nc = bacc.Bacc(
        "TRN2",
        target_bir_lowering=False,
        debug=False,
        enable_asserts=False,
        num_devices=N_CORES,
    )
    x_d = nc.dram_tensor("x", [NPC, CIN, H, W], F32, kind="ExternalInput")
    w_d = nc.dram_tensor("weight", [COUT, CIN, KH, KW], F32, kind="ExternalInput")
    b_d = nc.dram_tensor("bias", [COUT], F32, kind="ExternalInput")
    o_d = nc.dram_tensor("out", [NPC, COUT, H, W], F32, kind="ExternalOutput")
    ident_d = nc.inline_tensor(np.eye(128, dtype=ml_dtypes.bfloat16), name="ident")

    x_flat = x_d.ap().rearrange("n c h w -> n c (h w)")
    o_flat = o_d.ap().rearrange("n c h w -> n c (h w)")
    w_flat = w_d.ap().rearrange("o c kh kw -> o (c kh kw)")  # free idx = c*9 + tap

    with tile.TileContext(nc) as tc:
        with tc.tile_pool(name="persist", bufs=1) as pp, \
             tc.tile_pool(name="dram", bufs=1, space="DRAM") as dram:
            # ---- persistent tiles ----
            qx = [pp.tile([128, QCOLS], BF16, name=f"qx{i}") for i in range(NPC * 2)]
            # 36 weight tiles; idx = ct*18 + ot*9 + tap; scaled in place post-CC
            lhsT = pp.tile([128, 36, 128], BF16, name="lhsT")
            ident_sb = pp.tile([128, 128], BF16, name="ident_sb")
            # all small scalars packed into one tile (slots are 4KB-padded)
            misc = pp.tile([128, 160], F32, name="misc")
            ones_m = misc[0:1, 0:128]
            ones_k = misc[:, 128:129]
            bias_sb = misc[:, 130:132]
            wsb = misc[:, 132:134]     # col0 = ws, col1 = 1/ws
            xs = misc[:, 134:136]      # 127/amax, per ct half
            sc = misc[:, 136:138]      # ws*amax/127, per ct half
            amax2 = misc[:, 138:140]
            # partial amax: ct0 at cols 0..4 (n0,n1,n2,n3-half,n3-half),
            # ct1 at cols 5..9; the last image of each half is reduced in
            # halves so only a half-reduce trails its DMA
            pamax = misc[:, 140:150]
            ws1 = misc[0:1, 150:152]
            absw = misc[:, 152:154]
            negmagic = misc[:, 154:155]
            cc_in = [dram.tile([128, 1], F32, name=f"cc_in{i}") for i in range(2)]
            cc_out = [dram.tile([128, 1], F32, name=f"cc_out{i}",
                                addr_space="Shared") for i in range(2)]

            # ---- weights + constants first on the Act HW queue: the x
            # stream owns the SP queue, and HBM is idle for the first ~10us
            # while the rings spin up, so this is free bandwidth ----
            wt_scope = tc.tile_pool(name="wtmp", bufs=1)
            wp = wt_scope.__enter__()
            wt1 = []
            wdma = []
            for ot in range(2):
                wt = wp.tile([128, CIN * 9], F32, name=f"wt{ot}", tag=f"wt{ot}")
                wdma.append(nc.scalar.dma_start(wt[:],
                                                w_flat[ot * 128:(ot + 1) * 128, :]))
                wt1.append(wt)
            ident_dma = nc.scalar.dma_start(ident_sb[:], ident_d.ap())
            bias_dma = nc.scalar.dma_start(bias_sb,
                                           b_d.ap().rearrange("(o p) -> p o",
                                                              p=128))
            nc.vector.memset(ones_k, 1.0)
            nc.vector.memset(ones_m, 1.0)
            nc.vector.memset(negmagic, -MAGIC)

            # ---- pass A: stream x on the Sync HW queue, ct-major so the
            # ct0 collective can trigger before the ct1 tiles land. Every
            # tile stays resident in SBUF (~98KB/partition): no reloads. ----
            xres = {}
            for ct in range(2):
                for n in range(NPC):
                    t = pp.tile([128, HW], F32, name=f"xa{n}_{ct}")
                    src = x_flat[n, ct * 128:(ct + 1) * 128, :]
                    if n == NPC - 1:
                        # split the last tile of the half so only a
                        # half-reduce trails the collective trigger
                        nc.sync.dma_start(t[:, 0:HW // 2], src[:, 0:HW // 2])
                        nc.vector.reduce_max(pamax[:, 5 * ct + 3:5 * ct + 4],
                                             t[:, 0:HW // 2], axis=AX.X,
                                             apply_absolute_value=True)
                        nc.sync.dma_start(t[:, HW // 2:], src[:, HW // 2:])
                        nc.vector.reduce_max(pamax[:, 5 * ct + 4:5 * ct + 5],
                                             t[:, HW // 2:], axis=AX.X,
                                             apply_absolute_value=True)
                    else:
                        nc.sync.dma_start(t[:], src)
                        nc.vector.reduce_max(pamax[:, 5 * ct + n:5 * ct + n + 1],
                                             t[:], axis=AX.X,
                                             apply_absolute_value=True)
                    xres[(n, ct)] = t
                if ct == 0:
                    am0 = nc.vector.reduce_max(amax2[:, 0:1], pamax[:, 0:5],
                                               axis=AX.X)
                else:
                    am1 = nc.vector.reduce_max(amax2[:, 1:2], pamax[:, 5:10],
                                               axis=AX.X)

            # ---- qx zero-fill: only the padding cells are ever read and
            # never overwritten, so memset just those (head row, the
            # per-row wrap column, tail) on gpsimd ----
            for i in range(NPC * 2):
                nc.gpsimd.memset(qx[i][:, 0:PW + 1], 0.0)
                nc.gpsimd.memset(
                    qx[i][:, PW * 2:PW * 2 + H * PW].rearrange(
                        "p (h w) -> p h w", w=PW)[:, :, 0:1],
                    0.0,
                )
                nc.gpsimd.memset(qx[i][:, PW * 2 + H * PW - PW:QCOLS], 0.0)

            with tc.tile_pool(name="psum_t", bufs=4, space="PSUM") as pt_pool, \
                 tc.tile_pool(name="psum_s", bufs=1, space="PSUM") as ps_pool:
                # ---- ws = max(mean|w|,eps): per-partition |w|-sums on
                # vector (gpsimd elementwise measured ~17x slower -- never
                # offload bulk math there), total via PE with ones ----
                for ot in range(2):
                    ar = nc.vector.reduce_sum(absw[:, ot:ot + 1], wt1[ot][:],
                                              axis=AX.X,
                                              apply_absolute_value=True)
                    # both reduces, else the scheduler hoists the dep-free
                    # one to the queue front (bit us in v3.1): the weight
                    # math slots in after the ct0-amax, under CC0's shadow
                    add_dep_helper(ar.ins, am0.ins,
                                   reason="absw after ct0 amax")
                nc.vector.tensor_add(absw[:, 0:1], absw[:, 0:1], absw[:, 1:2])
                ps_s = ps_pool.tile([1, 1], F32, name="ps_s")
                nc.tensor.matmul(ps_s[:], ones_k, absw[:, 0:1], start=True,
                                 stop=True)
                nc.vector.tensor_scalar(ws1[:, 0:1], ps_s[:], 1.0 / FAN, EPS,
                                        op0=ALU.mult, op1=ALU.max)
                nc.vector.reciprocal(ws1[:, 1:2], ws1[:, 0:1])
                ps_b = ps_pool.tile([128, 2], F32, name="ps_b")
                nc.tensor.matmul(ps_b[:], ones_m, ws1[:, :], start=True, stop=True)
                wsb_cp = nc.scalar.copy(wsb, ps_b[:])

                # ---- collectives: input hop rides the Act HW queue (fast
                # completion), trigger from gpsimd (required engine). The
                # ct0 collective fires ~13us before the full amax would be
                # ready; CC1 follows as soon as the x stream drains. ----
                d0 = nc.scalar.dma_start(cc_in[0][:], amax2[:, 0:1])
                # d0 parks the in-order ACT queue until the ct0 amax lands:
                # it must sit after the weight/const descriptor writes (which
                # feed the whole weight pipeline) but NOT after the ws math
                for dep in (wdma[0], wdma[1], ident_dma, bias_dma):
                    add_dep_helper(d0.ins, dep.ins,
                                   reason="cc hop after const descriptors")
                cc0 = nc.gpsimd.collective_compute(
                    "AllReduce", ALU.max,
                    replica_groups=[list(range(N_CORES))],
                    ins=[cc_in[0].opt()], outs=[cc_out[0].opt()],
                )
                d1 = nc.scalar.dma_start(cc_in[1][:], amax2[:, 1:2])
                cc1 = nc.gpsimd.collective_compute(
                    "AllReduce", ALU.max,
                    replica_groups=[list(range(N_CORES))],
                    ins=[cc_in[1].opt()], outs=[cc_out[1].opt()],
                )
                r0 = nc.scalar.dma_start(amax2[:, 0:1], cc_out[0][:])
                r1 = nc.scalar.dma_start(amax2[:, 1:2], cc_out[1][:])

                # ---- ternary quantize qw = clip(round(w/ws), -1, 1) on
                # vector, ordered AFTER the amax-critical reduces (it's
                # local data, needed no earlier than CC0's return), then
                # PE-transpose each 128x128 block ----
                for ot in range(2):
                    wt = wt1[ot]
                    q1 = nc.vector.tensor_scalar(wt[:], wt[:], wsb[:, 1:2], MAGIC,
                                                 op0=ALU.mult, op1=ALU.add)
                    add_dep_helper(q1.ins, am0.ins,
                                   reason="ternarize after ct0 amax")
                    nc.vector.tensor_scalar_sub(wt[:], wt[:], MAGIC)
                    qwb = wp.tile([128, CIN * 9], BF16, name="qwb", tag="qwb",
                                  bufs=2)
                    nc.vector.tensor_scalar(qwb[:], wt[:], -1.0, 1.0,
                                            op0=ALU.max, op1=ALU.min)
                    wv = qwb.rearrange("p (c t) -> p t c", t=9)
                    for ct in range(2):
                        for tap in range(9):
                            idx = ct * 18 + ot * 9 + tap
                            pt = pt_pool.tile([128, 128], BF16, name="pt", tag="pt")
                            nc.tensor.transpose(
                                pt[:],
                                wv[:, tap, ct * 128:(ct + 1) * 128],
                                ident_sb[:],
                            )
                            nc.scalar.copy(lhsT[:, idx, :], pt[:])
            wt_scope.__exit__(None, None, None)

            # ---- post-CC0: scales for the ct0 half; fold s_c into lhsT ----
            x0 = nc.vector.tensor_scalar_max(amax2[:, 0:1], amax2[:, 0:1], EPS)
            # anti-hoist: the in-order vector queue must finish the amax
            # reduces before parking on the CC0-gated ops
            add_dep_helper(x0.ins, am1.ins, reason="post-cc0 after amax reduces")
            nc.vector.reciprocal(xs[:, 0:1], amax2[:, 0:1])
            nc.vector.tensor_scalar_mul(xs[:, 0:1], xs[:, 0:1], 127.0)
            nc.vector.tensor_scalar(sc[:, 0:1], amax2[:, 0:1], wsb[:, 0:1],
                                    1.0 / 127.0, op0=ALU.mult, op1=ALU.mult)

            def scale_lhsT(ct, ot):
                nc.vector.tensor_scalar_mul(
                    lhsT[:, ct * 18 + ot * 9:ct * 18 + (ot + 1) * 9, :],
                    lhsT[:, ct * 18 + ot * 9:ct * 18 + (ot + 1) * 9, :],
                    sc[:, ct:ct + 1],
                )

            def quantize(n, ct, quarters=False):
                # qx = round(x*xs): vector does x*xs+MAGIC in place (fp32),
                # ACT writes qx = t - MAGIC (exact, integer-valued bf16)
                t = xres[(n, ct)]
                tv = t.rearrange("p (h w) -> p h w", w=W)
                qxa = qx[n * 2 + ct][:, PW + 1:PW + 1 + H * PW].rearrange(
                    "p (h w) -> p h w", w=PW)[:, :, 0:W]
                nh = 4 if quarters else 1
                rh = H // nh
                vop = aop = None
                for hh in range(nh):
                    rs = slice(hh * rh, (hh + 1) * rh)
                    vop = nc.vector.tensor_scalar(
                        tv[:, rs, :], tv[:, rs, :],
                        xs[:, ct:ct + 1], MAGIC,
                        op0=ALU.mult, op1=ALU.add)
                    aop = nc.scalar.activation(
                        qxa[:, rs, :], tv[:, rs, :],
                        AF.Identity, bias=negmagic)
                return vop, aop

            scale_lhsT(0, 0)
            q0v, q0a = quantize(0, 0, quarters=True)
            scale_lhsT(0, 1)
            for n in range(1, NPC):
                quantize(n, 0)
            # the CC1 return hop must not park the in-order ACT queue ahead
            # of the first image's ct0 epilogues (they gate the first MM)
            add_dep_helper(r1.ins, q0a.ins,
                           reason="cc1 return after gating ct0 epilogue")

            # ---- post-CC1 scales (vector parks here while the ct0-ahead
            # conv groups below keep the PE busy) ----
            x1 = nc.vector.tensor_scalar_max(amax2[:, 1:2], amax2[:, 1:2], EPS)
            add_dep_helper(x1.ins, q0v.ins,
                           reason="post-cc1 after gating ct0 quantize")
            nc.vector.reciprocal(xs[:, 1:2], amax2[:, 1:2])
            nc.vector.tensor_scalar_mul(xs[:, 1:2], xs[:, 1:2], 127.0)
            nc.vector.tensor_scalar(sc[:, 1:2], amax2[:, 1:2], wsb[:, 0:1],
                                    1.0 / 127.0, op0=ALU.mult, op1=ALU.mult)
            scale_lhsT(1, 0)
            quantize(0, 1, quarters=True)
            scale_lhsT(1, 1)

            # ---- conv: 56 (image, cout-tile, chunk) groups, one PSUM bank
            # each. A group opens with its 9 ct0 taps and closes with the 9
            # ct1 taps + copy-out. AHEAD groups open on ct0 alone, so the
            # PE streams from CC0-land while CC1 is still in flight. ----
            def group_of(k):
                nk, rem = divmod(k, 2 * NCHUNK)
                ot, c8 = divmod(rem, NCHUNK)
                return nk, ot, c8

            def conv_half(ps, k, ct, start):
                nk, ot, c8 = group_of(k)
                base = c8 * CHUNK
                for tap in range(9):
                    di, dj = tap // 3, tap % 3
                    off = base + di * PW + dj
                    nc.tensor.matmul(
                        ps[:, 0:CHUNK],
                        lhsT[:, ct * 18 + ot * 9 + tap, :],
                        qx[nk * 2 + ct][:, off:off + CHUNK],
                        start=(start and tap == 0),
                        stop=(not start and tap == 8),
                        skip_group_check=True,
                    )

            with tc.tile_pool(name="psum_c", bufs=8, space="PSUM") as pc_pool, \
                 tc.tile_pool(name="outp", bufs=6) as op_pool:
                open_ps = {}
                for j in range(AHEAD):
                    open_ps[j] = pc_pool.tile([128, 512], F32, name="ps", tag="ps")
                    conv_half(open_ps[j], j, 0, True)
                for k in range(NGRP):
                    if k % (2 * NCHUNK) == 0 and k // (2 * NCHUNK) + 1 < NPC:
                        quantize(k // (2 * NCHUNK) + 1, 1)
                    ps = open_ps.pop(k)
                    conv_half(ps, k, 1, False)
                    nk, ot, c8 = group_of(k)
                    ob = op_pool.tile([128, OUT_CHUNK], F32, name="ob", tag="ob")
                    nc.scalar.activation(
                        ob.rearrange("p (h w) -> p h w", w=W),
                        ps[:, 0:CHUNK].rearrange(
                            "p (h w) -> p h w", w=PW)[:, :, 0:W],
                        AF.Identity, bias=bias_sb[:, ot:ot + 1])
                    nc.sync.dma_start(
                        o_flat[nk, ot * 128:(ot + 1) * 128,
                               c8 * OUT_CHUNK:(c8 + 1) * OUT_CHUNK],
                        ob[:],
                    )
                    j = k + AHEAD
                    if j < NGRP:
                        open_ps[j] = pc_pool.tile([128, 512], F32,
                                                  name="ps", tag="ps")
                        conv_half(open_ps[j], j, 0, True)

    nc.compile()
    return nc


_NC_CACHE = None


def _get_program():
    global _NC_CACHE
    if _NC_CACHE is None:
        _NC_CACHE = _build_program()
    return _NC_CACHE


def _install_ntff_hook():
    """Register the axon NTFF profiling hook (the antenv stub lacks it)."""
    try:
        import antenv
        if getattr(antenv, "axon_hooks", None) is not None:
            return
        mod = types.ModuleType("antenv.axon_hooks")
        mod._hook = None
        def set_axon_ntff_profile_hook(h):
            mod._hook = h
        def get_axon_ntff_profile_hook():
            return mod._hook
        mod.set_axon_ntff_profile_hook = set_axon_ntff_profile_hook
        mod.get_axon_ntff_profile_hook = get_axon_ntff_profile_hook
        sys.modules["antenv.axon_hooks"] = mod
        antenv.axon_hooks = mod
        from trn_agent_boot.trn_boot import _ntff_profile_via_ctypes
        set_axon_ntff_profile_hook(_ntff_profile_via_ctypes("/opt/axon/libaxon_pjrt.so"))
    except Exception:
        pass


def run(x, weight, bias, trace=False, trace_cores=None):
    x = np.ascontiguousarray(np.asarray(x, dtype=np.float32))
    weight = np.ascontiguousarray(np.asarray(weight, dtype=np.float32))
    bias = np.ascontiguousarray(np.asarray(bias, dtype=np.float32))
    assert x.shape == (N, CIN, H, W), x.shape
    nc = _get_program()
    in_maps = [
        {"x": x[c * NPC:(c + 1) * NPC], "weight": weight, "bias": bias}
        for c in range(N_CORES)
    ]
    if trace:
        _install_ntff_hook()
    res = run_bass_kernel_spmd(nc, in_maps, list(range(N_CORES)), trace=trace,
                               trace_cores=trace_cores)
    out = np.concatenate([res.results[c]["out"] for c in range(N_CORES)], axis=0)
    return out, res


def kernel(x, weight, bias):
    out, _ = run(x, weight, bias, trace=False)
    return out


# revision 19
# speedup vs baseline: 1.0640x; 1.0168x over previous
"""BitConv2d (BitNet-style fake-quant 3x3 conv) Trainium2 Bass kernel.

Reference computation (see problem):
  ws   = max(mean|w|, 1e-6);  qw = clip(round(w/ws), -1, 1)           (per-tensor ternary)
  amax = max(max|x| over (N,H,W) per channel, 1e-6); xs = 127/amax
  qx   = clip(round(x*xs), -128, 127)                                  (per-channel int8)
  out  = conv2d(qx/xs, qw*ws, stride 1, pad 1, NCHW/OIHW) + bias

Key algebraic restructuring for the tensor engine:
  out[n,o,h,w] = sum_{c,i,j} qx[n,c,h+i-1,w+j-1] * (qw[o,c,i,j] * ws * amax[c]/127)
so the conv runs as bf16 matmuls with
  rhs  = qx          (integers in [-127,127]  -> EXACT in bf16)
  lhsT = qw * s_c    (ternary * per-in-channel scale, bf16-rounded once per channel)
accumulated in fp32 PSUM. The 3x3 conv is 18 accumulating matmuls
(2 cin-tiles x 9 taps) over a zero-padded flat spatial layout with row
stride 57 (one left-pad column per row doubles as the previous row's right
pad), where each tap is a constant flat column offset di*57+dj.

Sharding: data-parallel over batch (4 images/core on 8 cores), weight
replicated (ws computed redundantly); per-channel amax needs a global max.

v3 startup pipeline: the global amax AllReduce is SPLIT per cin-half.
x streams ct-major (all images' channels 0..127 first), so CC0 (amax of
ct0) triggers at ~2/3 of the x-load time and CC1 right after the load.
All weight prep (ws, ternarize on gpsimd, PE transposes) happens under
CC0's latency using only local data. When CC0 lands, the conv starts on
ct0-only accumulation groups across 7 PSUM banks (~15us of PE work)
while CC1 is still in flight; each group is closed by its 9 ct1 taps
once CC1's scales arrive. All of x stays resident in SBUF (no reloads);
x owns the SP hardware DMA queue, weights/collective hops ride the
Activation hardware queue (the gpsimd software queue's ~8us completion
latency would sit on the collective critical path).
"""

import sys
import types

for _p in ("/opt/trn_rl_repo", "/root/.axon_site/_ro/trn_rl_repo"):
    if _p not in sys.path:
        sys.path.insert(0, _p)

import numpy as np
import ml_dtypes

import concourse.bacc as bacc
import concourse.mybir as mybir
import concourse.tile as tile
from concourse.bass_utils import run_bass_kernel_spmd
from concourse.tile_rust import add_dep_helper

F32 = mybir.dt.float32
BF16 = mybir.dt.bfloat16
ALU = mybir.AluOpType
AX = mybir.AxisListType
AF = mybir.ActivationFunctionType

N_CORES = 8
N, CIN, H, W = 32, 256, 56, 56
COUT, KH, KW = 256, 3, 3
NPC = N // N_CORES          # images per core
HW = H * W                  # 3136
PW = W + 1                  # 57: padded row stride (left pad doubles as right pad)
QCOLS = 3312                # >= (55+2)*57 + 58 = 3307, 8-aligned
ROWS_PER_CHUNK = 8
CHUNK = ROWS_PER_CHUNK * PW   # 456 psum cols per chunk (<=512, one bank)
NCHUNK = H // ROWS_PER_CHUNK  # 7
OUT_CHUNK = ROWS_PER_CHUNK * W  # 448 valid cols per chunk
MAGIC = 12582912.0          # 1.5*2^23: (v+MAGIC)-MAGIC == round-half-even(v)
EPS = 1e-6
FAN = COUT * CIN * KH * KW  # weight element count for mean|w|
NGRP = NPC * 2 * NCHUNK     # 56 (image, cout-tile, chunk) psum groups
AHEAD = 7                   # ct0-only groups in flight while CC1 is pending


def _build_program():
    nc = bacc.Bacc(
        "TRN2",
        target_bir_lowering=False,
        debug=False,
        enable_asserts=False,
        num_devices=N_CORES,
    )
    x_d = nc.dram_tensor("x", [NPC, CIN, H, W], F32, kind="ExternalInput")
    w_d = nc.dram_tensor("weight", [COUT, CIN, KH, KW], F32, kind="ExternalInput")
    b_d = nc.dram_tensor("bias", [COUT], F32, kind="ExternalInput")
    o_d = nc.dram_tensor("out", [NPC, COUT, H, W], F32, kind="ExternalOutput")
    ident_d = nc.inline_tensor(np.eye(128, dtype=ml_dtypes.bfloat16), name="ident")

    x_flat = x_d.ap().rearrange("n c h w -> n c (h w)")
    o_flat = o_d.ap().rearrange("n c h w -> n c (h w)")
    w_flat = w_d.ap().rearrange("o c kh kw -> o (c kh kw)")  # free idx = c*9 + tap

    with tile.TileContext(nc) as tc:
        with tc.tile_pool(name="persist", bufs=1) as pp, \
             tc.tile_pool(name="dram", bufs=1, space="DRAM") as dram:
            # ---- persistent tiles ----
            qx = [pp.tile([128, QCOLS], BF16, name=f"qx{i}") for i in range(NPC * 2)]
            # 36 weight tiles; idx = ct*18 + ot*9 + tap; scaled in place post-CC
            lhsT = pp.tile([128, 36, 128], BF16, name="lhsT")
            ident_sb = pp.tile([128, 128], BF16, name="ident_sb")
            # all small scalars packed into one tile (slots are 4KB-padded)
            misc = pp.tile([128, 160], F32, name="misc")
            ones_m = misc[0:1, 0:128]
            ones_k = misc[:, 128:129]
            bias_sb = misc[:, 130:132]
            wsb = misc[:, 132:134]     # col0 = ws, col1 = 1/ws
            xs = misc[:, 134:136]      # 127/amax, per ct half
            sc = misc[:, 136:138]      # ws*amax/127, per ct half
            amax2 = misc[:, 138:140]
            # partial amax: ct0 at cols 0..4 (n0,n1,n2,n3-half,n3-half),
            # ct1 at cols 5..9; the last image of each half is reduced in
            # halves so only a half-reduce trails its DMA
            pamax = misc[:, 140:150]
            ws1 = misc[0:1, 150:152]
            absw = misc[:, 152:154]
            negmagic = misc[:, 154:155]
            cc_in = dram.tile([128, 2], F32, name="cc_in")
            cc_out = dram.tile([128, 2], F32, name="cc_out",
                               addr_space="Shared")

            # ---- weights + constants first on the Act HW queue: the x
            # stream owns the SP queue, and HBM is idle for the first ~10us
            # while the rings spin up, so this is free bandwidth ----
            wt_scope = tc.tile_pool(name="wtmp", bufs=1)
            wp = wt_scope.__enter__()
            wt1 = []
            wdma = []
            for ot in range(2):
                wt = wp.tile([128, CIN * 9], F32, name=f"wt{ot}", tag=f"wt{ot}")
                wdma.append(nc.scalar.dma_start(wt[:],
                                                w_flat[ot * 128:(ot + 1) * 128, :]))
                wt1.append(wt)
            ident_dma = nc.scalar.dma_start(ident_sb[:], ident_d.ap())
            bias_dma = nc.scalar.dma_start(bias_sb,
                                           b_d.ap().rearrange("(o p) -> p o",
                                                              p=128))
            nc.vector.memset(ones_k, 1.0)
            nc.vector.memset(ones_m, 1.0)
            nc.vector.memset(negmagic, -MAGIC)

            # ---- pass A: stream x on the Sync HW queue, ct-major so the
            # ct0 collective can trigger before the ct1 tiles land. Every
            # tile stays resident in SBUF (~98KB/partition): no reloads. ----
            xres = {}
            for ct in range(2):
                for n in range(NPC):
                    t = pp.tile([128, HW], F32, name=f"xa{n}_{ct}")
                    src = x_flat[n, ct * 128:(ct + 1) * 128, :]
                    if n == NPC - 1:
                        # split the last tile of the half so only a
                        # half-reduce trails the collective trigger
                        nc.sync.dma_start(t[:, 0:HW // 2], src[:, 0:HW // 2])
                        nc.vector.reduce_max(pamax[:, 5 * ct + 3:5 * ct + 4],
                                             t[:, 0:HW // 2], axis=AX.X,
                                             apply_absolute_value=True)
                        nc.sync.dma_start(t[:, HW // 2:], src[:, HW // 2:])
                        nc.vector.reduce_max(pamax[:, 5 * ct + 4:5 * ct + 5],
                                             t[:, HW // 2:], axis=AX.X,
                                             apply_absolute_value=True)
                    else:
                        nc.sync.dma_start(t[:], src)
                        nc.vector.reduce_max(pamax[:, 5 * ct + n:5 * ct + n + 1],
                                             t[:], axis=AX.X,
                                             apply_absolute_value=True)
                    xres[(n, ct)] = t
                if ct == 0:
                    am0 = nc.vector.reduce_max(amax2[:, 0:1], pamax[:, 0:5],
                                               axis=AX.X)
                else:
                    am1 = nc.vector.reduce_max(amax2[:, 1:2], pamax[:, 5:10],
                                               axis=AX.X)

            # ---- qx zero-fill: only the padding cells are ever read and
            # never overwritten, so memset just those (head row, the
            # per-row wrap column, tail) on gpsimd ----
            for i in range(NPC * 2):
                nc.gpsimd.memset(qx[i][:, 0:PW + 1], 0.0)
                nc.gpsimd.memset(
                    qx[i][:, PW * 2:PW * 2 + H * PW].rearrange(
                        "p (h w) -> p h w", w=PW)[:, :, 0:1],
                    0.0,
                )
                nc.gpsimd.memset(qx[i][:, PW * 2 + H * PW - PW:QCOLS], 0.0)

            with tc.tile_pool(name="psum_t", bufs=4, space="PSUM") as pt_pool, \
                 tc.tile_pool(name="psum_s", bufs=1, space="PSUM") as ps_pool:
                # ---- ws = max(mean|w|,eps): per-partition |w|-sums on
                # vector (gpsimd elementwise measured ~17x slower -- never
                # offload bulk math there), total via PE with ones ----
                for ot in range(2):
                    ar = nc.vector.reduce_sum(absw[:, ot:ot + 1], wt1[ot][:],
                                              axis=AX.X,
                                              apply_absolute_value=True)
                    # both reduces, else the scheduler hoists the dep-free
                    # one to the queue front (bit us in v3.1): the weight
                    # math slots in after the ct0-amax, under CC0's shadow
                    add_dep_helper(ar.ins, am0.ins,
                                   reason="absw after ct0 amax")
                nc.vector.tensor_add(absw[:, 0:1], absw[:, 0:1], absw[:, 1:2])
                ps_s = ps_pool.tile([1, 1], F32, name="ps_s")
                nc.tensor.matmul(ps_s[:], ones_k, absw[:, 0:1], start=True,
                                 stop=True)
                nc.vector.tensor_scalar(ws1[:, 0:1], ps_s[:], 1.0 / FAN, EPS,
                                        op0=ALU.mult, op1=ALU.max)
                nc.vector.reciprocal(ws1[:, 1:2], ws1[:, 0:1])
                ps_b = ps_pool.tile([128, 2], F32, name="ps_b")
                nc.tensor.matmul(ps_b[:], ones_m, ws1[:, :], start=True, stop=True)
                wsb_cp = nc.scalar.copy(wsb, ps_b[:])

                # ---- the amax collective: one [128,2] AllReduce (a split
                # per-ct pair measured ~25us SLOWER end-to-end on the first
                # op, erasing the early-trigger win). Input hop rides the
                # Act HW queue (fast completion), trigger from gpsimd. ----
                d0 = nc.scalar.dma_start(cc_in[:], amax2)
                # d0 parks the in-order ACT queue until the amax lands: it
                # must sit after the weight/const descriptor writes (which
                # feed the whole weight pipeline) but NOT after the ws math
                for dep in (wdma[0], wdma[1], ident_dma, bias_dma):
                    add_dep_helper(d0.ins, dep.ins,
                                   reason="cc hop after const descriptors")
                nc.gpsimd.collective_compute(
                    "AllReduce", ALU.max,
                    replica_groups=[list(range(N_CORES))],
                    ins=[cc_in.opt()], outs=[cc_out.opt()],
                )
                r0 = nc.scalar.dma_start(amax2, cc_out[:])
                r1 = r0

                # ---- ternary quantize qw = clip(round(w/ws), -1, 1) on
                # vector, ordered AFTER the amax-critical reduces (it's
                # local data, needed no earlier than CC0's return), then
                # PE-transpose each 128x128 block ----
                for ot in range(2):
                    wt = wt1[ot]
                    q1 = nc.vector.tensor_scalar(wt[:], wt[:], wsb[:, 1:2], MAGIC,
                                                 op0=ALU.mult, op1=ALU.add)
                    add_dep_helper(q1.ins, am0.ins,
                                   reason="ternarize after ct0 amax")
                    nc.vector.tensor_scalar_sub(wt[:], wt[:], MAGIC)
                    qwb = wp.tile([128, CIN * 9], BF16, name="qwb", tag="qwb",
                                  bufs=2)
                    nc.vector.tensor_scalar(qwb[:], wt[:], -1.0, 1.0,
                                            op0=ALU.max, op1=ALU.min)
                    wv = qwb.rearrange("p (c t) -> p t c", t=9)
                    for ct in range(2):
                        for tap in range(9):
                            idx = ct * 18 + ot * 9 + tap
                            pt = pt_pool.tile([128, 128], BF16, name="pt", tag="pt")
                            nc.tensor.transpose(
                                pt[:],
                                wv[:, tap, ct * 128:(ct + 1) * 128],
                                ident_sb[:],
                            )
                            nc.scalar.copy(lhsT[:, idx, :], pt[:])
            wt_scope.__exit__(None, None, None)

            # ---- post-CC0: scales for the ct0 half; fold s_c into lhsT ----
            x0 = nc.vector.tensor_scalar_max(amax2[:, 0:1], amax2[:, 0:1], EPS)
            # anti-hoist: the in-order vector queue must finish the amax
            # reduces before parking on the CC0-gated ops
            add_dep_helper(x0.ins, am1.ins, reason="post-cc0 after amax reduces")
            nc.vector.reciprocal(xs[:, 0:1], amax2[:, 0:1])
            nc.vector.tensor_scalar_mul(xs[:, 0:1], xs[:, 0:1], 127.0)
            nc.vector.tensor_scalar(sc[:, 0:1], amax2[:, 0:1], wsb[:, 0:1],
                                    1.0 / 127.0, op0=ALU.mult, op1=ALU.mult)

            def scale_lhsT(ct, ot):
                nc.vector.tensor_scalar_mul(
                    lhsT[:, ct * 18 + ot * 9:ct * 18 + (ot + 1) * 9, :],
                    lhsT[:, ct * 18 + ot * 9:ct * 18 + (ot + 1) * 9, :],
                    sc[:, ct:ct + 1],
                )

            def quantize(n, ct, quarters=False):
                # qx = round(x*xs): vector does x*xs+MAGIC in place (fp32),
                # ACT writes qx = t - MAGIC (exact, integer-valued bf16)
                t = xres[(n, ct)]
                tv = t.rearrange("p (h w) -> p h w", w=W)
                qxa = qx[n * 2 + ct][:, PW + 1:PW + 1 + H * PW].rearrange(
                    "p (h w) -> p h w", w=PW)[:, :, 0:W]
                nh = 4 if quarters else 1
                rh = H // nh
                vop = aop = None
                for hh in range(nh):
                    rs = slice(hh * rh, (hh + 1) * rh)
                    vop = nc.vector.tensor_scalar(
                        tv[:, rs, :], tv[:, rs, :],
                        xs[:, ct:ct + 1], MAGIC,
                        op0=ALU.mult, op1=ALU.add)
                    aop = nc.scalar.activation(
                        qxa[:, rs, :], tv[:, rs, :],
                        AF.Identity, bias=negmagic)
                return vop, aop

            scale_lhsT(0, 0)
            q0v, q0a = quantize(0, 0, quarters=True)
            scale_lhsT(0, 1)
            for n in range(1, NPC):
                quantize(n, 0)

            # ---- post-CC1 scales (vector parks here while the ct0-ahead
            # conv groups below keep the PE busy) ----
            x1 = nc.vector.tensor_scalar_max(amax2[:, 1:2], amax2[:, 1:2], EPS)
            add_dep_helper(x1.ins, q0v.ins,
                           reason="post-cc1 after gating ct0 quantize")
            nc.vector.reciprocal(xs[:, 1:2], amax2[:, 1:2])
            nc.vector.tensor_scalar_mul(xs[:, 1:2], xs[:, 1:2], 127.0)
            nc.vector.tensor_scalar(sc[:, 1:2], amax2[:, 1:2], wsb[:, 0:1],
                                    1.0 / 127.0, op0=ALU.mult, op1=ALU.mult)
            scale_lhsT(1, 0)
            quantize(0, 1, quarters=True)
            scale_lhsT(1, 1)

            # ---- conv: 56 (image, cout-tile, chunk) groups, one PSUM bank
            # each. A group opens with its 9 ct0 taps and closes with the 9
            # ct1 taps + copy-out. AHEAD groups open on ct0 alone, so the
            # PE streams from CC0-land while CC1 is still in flight. ----
            def group_of(k):
                nk, rem = divmod(k, 2 * NCHUNK)
                ot, c8 = divmod(rem, NCHUNK)
                return nk, ot, c8

            def conv_half(ps, k, ct, start):
                nk, ot, c8 = group_of(k)
                base = c8 * CHUNK
                for tap in range(9):
                    di, dj = tap // 3, tap % 3
                    off = base + di * PW + dj
                    nc.tensor.matmul(
                        ps[:, 0:CHUNK],
                        lhsT[:, ct * 18 + ot * 9 + tap, :],
                        qx[nk * 2 + ct][:, off:off + CHUNK],
                        start=(start and tap == 0),
                        stop=(not start and tap == 8),
                        skip_group_check=True,
                    )

            with tc.tile_pool(name="psum_c", bufs=8, space="PSUM") as pc_pool, \
                 tc.tile_pool(name="outp", bufs=6) as op_pool:
                open_ps = {}
                for j in range(AHEAD):
                    open_ps[j] = pc_pool.tile([128, 512], F32, name="ps", tag="ps")
                    conv_half(open_ps[j], j, 0, True)
                for k in range(NGRP):
                    if k % (2 * NCHUNK) == 0 and k // (2 * NCHUNK) + 1 < NPC:
                        quantize(k // (2 * NCHUNK) + 1, 1)
                    ps = open_ps.pop(k)
                    conv_half(ps, k, 1, False)
                    nk, ot, c8 = group_of(k)
                    ob = op_pool.tile([128, OUT_CHUNK], F32, name="ob", tag="ob")
                    nc.scalar.activation(
                        ob.rearrange("p (h w) -> p h w", w=W),
                        ps[:, 0:CHUNK].rearrange(
                            "p (h w) -> p h w", w=PW)[:, :, 0:W],
                        AF.Identity, bias=bias_sb[:, ot:ot + 1])
                    nc.sync.dma_start(
                        o_flat[nk, ot * 128:(ot + 1) * 128,
                               c8 * OUT_CHUNK:(c8 + 1) * OUT_CHUNK],
                        ob[:],
                    )
                    j = k + AHEAD
                    if j < NGRP:
                        open_ps[j] = pc_pool.tile([128, 512], F32,
                                                  name="ps", tag="ps")
                        conv_half(open_ps[j], j, 0, True)

    nc.compile()
    return nc


_NC_CACHE = None


def _get_program():
    global _NC_CACHE
    if _NC_CACHE is None:
        _NC_CACHE = _build_program()
    return _NC_CACHE


def _install_ntff_hook():
    """Register the axon NTFF profiling hook (the antenv stub lacks it)."""
    try:
        import antenv
        if getattr(antenv, "axon_hooks", None) is not None:
            return
        mod = types.ModuleType("antenv.axon_hooks")
        mod._hook = None
        def set_axon_ntff_profile_hook(h):
            mod._hook = h
        def get_axon_ntff_profile_hook():
            return mod._hook
        mod.set_axon_ntff_profile_hook = set_axon_ntff_profile_hook
        mod.get_axon_ntff_profile_hook = get_axon_ntff_profile_hook
        sys.modules["antenv.axon_hooks"] = mod
        antenv.axon_hooks = mod
        from trn_agent_boot.trn_boot import _ntff_profile_via_ctypes
        set_axon_ntff_profile_hook(_ntff_profile_via_ctypes("/opt/axon/libaxon_pjrt.so"))
    except Exception:
        pass


def run(x, weight, bias, trace=False, trace_cores=None):
    x = np.ascontiguousarray(np.asarray(x, dtype=np.float32))
    weight = np.ascontiguousarray(np.asarray(weight, dtype=np.float32))
    bias = np.ascontiguousarray(np.asarray(bias, dtype=np.float32))
    assert x.shape == (N, CIN, H, W), x.shape
    nc = _get_program()
    in_maps = [
        {"x": x[c * NPC:(c + 1) * NPC], "weight": weight, "bias": bias}
        for c in range(N_CORES)
    ]
    if trace:
        _install_ntff_hook()
    res = run_bass_kernel_spmd(nc, in_maps, list(range(N_CORES)), trace=trace,
                               trace_cores=trace_cores)
    out = np.concatenate([res.results[c]["out"] for c in range(N_CORES)], axis=0)
    return out, res


def kernel(x, weight, bias):
    out, _ = run(x, weight, bias, trace=False)
    return out


# revision 21
# speedup vs baseline: 1.0877x; 1.0223x over previous
"""BitConv2d (BitNet-style fake-quant 3x3 conv) Trainium2 Bass kernel.

Reference computation (see problem):
  ws   = max(mean|w|, 1e-6);  qw = clip(round(w/ws), -1, 1)           (per-tensor ternary)
  amax = max(max|x| over (N,H,W) per channel, 1e-6); xs = 127/amax
  qx   = clip(round(x*xs), -128, 127)                                  (per-channel int8)
  out  = conv2d(qx/xs, qw*ws, stride 1, pad 1, NCHW/OIHW) + bias

Key algebraic restructuring for the tensor engine:
  out[n,o,h,w] = sum_{c,i,j} qx[n,c,h+i-1,w+j-1] * (qw[o,c,i,j] * ws * amax[c]/127)
so the conv runs as bf16 matmuls with
  rhs  = qx          (integers in [-127,127]  -> EXACT in bf16)
  lhsT = qw * s_c    (ternary * per-in-channel scale, bf16-rounded once per channel)
accumulated in fp32 PSUM. The 3x3 conv is 18 accumulating matmuls
(2 cin-tiles x 9 taps) over a zero-padded flat spatial layout with row
stride 57 (one left-pad column per row doubles as the previous row's right
pad), where each tap is a constant flat column offset di*57+dj.

Sharding: data-parallel over batch (4 images/core on 8 cores), weight
replicated (ws computed redundantly); per-channel amax needs a global max.

Startup pipeline (v3.4): one [128,2] AllReduce of the per-channel amax
(splitting it per cin-half measured ~25us slower end-to-end on the first
op). The doorbell fires ~3us after the local amax: all weight prep (ws,
ternarize, PE transposes) is dependency-pinned BEHIND the amax reduces
on the in-order vector queue and runs inside the collective's ~30-50us
shadow. All of x stays resident in SBUF (no second pass); x owns the SP
hardware DMA queue from the first descriptor, weights/constants and the
collective's DRAM hops ride the Activation hardware queue (the gpsimd
software queue's ~8us completion latency would sit on the collective
critical path; gpsimd carries only the doorbell). The conv opens AHEAD
accumulation groups (one PSUM bank each: 9 ct0 taps) before closing
each group with its 9 ct1 taps + biased copy-out, which keeps the PE
streaming back-to-back at the throttled-clock floor (234ns per 456-col
matmul at 13/16 x 2.4GHz) with zero mid-stream stalls.
"""

import sys
import types

for _p in ("/opt/trn_rl_repo", "/root/.axon_site/_ro/trn_rl_repo"):
    if _p not in sys.path:
        sys.path.insert(0, _p)

import numpy as np
import ml_dtypes

import concourse.bacc as bacc
import concourse.mybir as mybir
import concourse.tile as tile
from concourse.bass_utils import run_bass_kernel_spmd
from concourse.tile_rust import add_dep_helper

F32 = mybir.dt.float32
BF16 = mybir.dt.bfloat16
ALU = mybir.AluOpType
AX = mybir.AxisListType
AF = mybir.ActivationFunctionType

N_CORES = 8
N, CIN, H, W = 32, 256, 56, 56
COUT, KH, KW = 256, 3, 3
NPC = N // N_CORES          # images per core
HW = H * W                  # 3136
PW = W + 1                  # 57: padded row stride (left pad doubles as right pad)
QCOLS = 3312                # >= (55+2)*57 + 58 = 3307, 8-aligned
ROWS_PER_CHUNK = 8
CHUNK = ROWS_PER_CHUNK * PW   # 456 psum cols per chunk (<=512, one bank)
NCHUNK = H // ROWS_PER_CHUNK  # 7
OUT_CHUNK = ROWS_PER_CHUNK * W  # 448 valid cols per chunk
MAGIC = 12582912.0          # 1.5*2^23: (v+MAGIC)-MAGIC == round-half-even(v)
EPS = 1e-6
FAN = COUT * CIN * KH * KW  # weight element count for mean|w|
NGRP = NPC * 2 * NCHUNK     # 56 (image, cout-tile, chunk) psum groups
AHEAD = 7                   # ct0-only groups in flight while CC1 is pending


def _build_program():
    nc = bacc.Bacc(
        "TRN2",
        target_bir_lowering=False,
        debug=False,
        enable_asserts=False,
        num_devices=N_CORES,
    )
    x_d = nc.dram_tensor("x", [NPC, CIN, H, W], F32, kind="ExternalInput")
    w_d = nc.dram_tensor("weight", [COUT, CIN, KH, KW], F32, kind="ExternalInput")
    b_d = nc.dram_tensor("bias", [COUT], F32, kind="ExternalInput")
    o_d = nc.dram_tensor("out", [NPC, COUT, H, W], F32, kind="ExternalOutput")
    ident_d = nc.inline_tensor(np.eye(128, dtype=ml_dtypes.bfloat16), name="ident")

    x_flat = x_d.ap().rearrange("n c h w -> n c (h w)")
    o_flat = o_d.ap().rearrange("n c h w -> n c (h w)")
    w_flat = w_d.ap().rearrange("o c kh kw -> o (c kh kw)")  # free idx = c*9 + tap

    with tile.TileContext(nc) as tc:
        with tc.tile_pool(name="persist", bufs=1) as pp, \
             tc.tile_pool(name="dram", bufs=1, space="DRAM") as dram:
            # ---- persistent tiles ----
            qx = [pp.tile([128, QCOLS], BF16, name=f"qx{i}") for i in range(NPC * 2)]
            # 36 weight tiles; idx = ct*18 + ot*9 + tap; scaled in place post-CC
            lhsT = pp.tile([128, 36, 128], BF16, name="lhsT")
            ident_sb = pp.tile([128, 128], BF16, name="ident_sb")
            # all small scalars packed into one tile (slots are 4KB-padded)
            misc = pp.tile([128, 160], F32, name="misc")
            ones_m = misc[0:1, 0:128]
            ones_k = misc[:, 128:129]
            bias_sb = misc[:, 130:132]
            wsb = misc[:, 132:134]     # col0 = ws, col1 = 1/ws
            xs = misc[:, 134:136]      # 127/amax, per ct half
            sc = misc[:, 136:138]      # ws*amax/127, per ct half
            amax2 = misc[:, 138:140]
            # partial amax: ct0 at cols 0..4 (n0,n1,n2,n3-half,n3-half),
            # ct1 at cols 5..9; the last image of each half is reduced in
            # halves so only a half-reduce trails its DMA
            pamax = misc[:, 140:150]
            ws1 = misc[0:1, 150:152]
            absw = misc[:, 152:154]
            negmagic = misc[:, 154:155]
            cc_in = dram.tile([128, 2], F32, name="cc_in")
            cc_out = dram.tile([128, 2], F32, name="cc_out",
                               addr_space="Shared")

            # ---- weights + constants first on the Act HW queue: the x
            # stream owns the SP queue, and HBM is idle for the first ~10us
            # while the rings spin up, so this is free bandwidth ----
            wt_scope = tc.tile_pool(name="wtmp", bufs=1)
            wp = wt_scope.__enter__()
            wt1 = []
            wdma = []
            for ot in range(2):
                wt = wp.tile([128, CIN * 9], F32, name=f"wt{ot}", tag=f"wt{ot}")
                wdma.append(nc.scalar.dma_start(wt[:],
                                                w_flat[ot * 128:(ot + 1) * 128, :]))
                wt1.append(wt)
            ident_dma = nc.scalar.dma_start(ident_sb[:], ident_d.ap())
            bias_dma = nc.scalar.dma_start(bias_sb,
                                           b_d.ap().rearrange("(o p) -> p o",
                                                              p=128))
            nc.vector.memset(ones_k, 1.0)
            nc.vector.memset(ones_m, 1.0)
            nc.vector.memset(negmagic, -MAGIC)

            # ---- pass A: stream x on the Sync HW queue, ct-major so the
            # ct0 collective can trigger before the ct1 tiles land. Every
            # tile stays resident in SBUF (~98KB/partition): no reloads. ----
            xres = {}
            for ct in range(2):
                for n in range(NPC):
                    t = pp.tile([128, HW], F32, name=f"xa{n}_{ct}")
                    src = x_flat[n, ct * 128:(ct + 1) * 128, :]
                    if n == NPC - 1:
                        # split the last tile of the half so only a
                        # half-reduce trails the collective trigger
                        nc.sync.dma_start(t[:, 0:HW // 2], src[:, 0:HW // 2])
                        nc.vector.reduce_max(pamax[:, 5 * ct + 3:5 * ct + 4],
                                             t[:, 0:HW // 2], axis=AX.X,
                                             apply_absolute_value=True)
                        nc.sync.dma_start(t[:, HW // 2:], src[:, HW // 2:])
                        nc.vector.reduce_max(pamax[:, 5 * ct + 4:5 * ct + 5],
                                             t[:, HW // 2:], axis=AX.X,
                                             apply_absolute_value=True)
                    else:
                        nc.sync.dma_start(t[:], src)
                        nc.vector.reduce_max(pamax[:, 5 * ct + n:5 * ct + n + 1],
                                             t[:], axis=AX.X,
                                             apply_absolute_value=True)
                    xres[(n, ct)] = t
                if ct == 0:
                    am0 = nc.vector.reduce_max(amax2[:, 0:1], pamax[:, 0:5],
                                               axis=AX.X)
                else:
                    am1 = nc.vector.reduce_max(amax2[:, 1:2], pamax[:, 5:10],
                                               axis=AX.X)

            # ---- qx zero-fill: only the padding cells are ever read and
            # never overwritten, so memset just those (head row, the
            # per-row wrap column, tail) on gpsimd ----
            for i in range(NPC * 2):
                nc.gpsimd.memset(qx[i][:, 0:PW + 1], 0.0)
                nc.gpsimd.memset(
                    qx[i][:, PW * 2:PW * 2 + H * PW].rearrange(
                        "p (h w) -> p h w", w=PW)[:, :, 0:1],
                    0.0,
                )
                nc.gpsimd.memset(qx[i][:, PW * 2 + H * PW - PW:QCOLS], 0.0)

            with tc.tile_pool(name="psum_t", bufs=4, space="PSUM") as pt_pool, \
                 tc.tile_pool(name="psum_s", bufs=1, space="PSUM") as ps_pool:
                # ---- ws = max(mean|w|,eps): per-partition |w|-sums on
                # vector (gpsimd elementwise measured ~17x slower -- never
                # offload bulk math there), total via PE with ones ----
                for ot in range(2):
                    ar = nc.vector.reduce_sum(absw[:, ot:ot + 1], wt1[ot][:],
                                              axis=AX.X,
                                              apply_absolute_value=True)
                    # both reduces, else the scheduler hoists the dep-free
                    # one to the queue front (bit us in v3.1): the weight
                    # math slots in after the ct0-amax, under CC0's shadow
                    add_dep_helper(ar.ins, am1.ins,
                                   reason="absw after the full amax")
                nc.vector.tensor_add(absw[:, 0:1], absw[:, 0:1], absw[:, 1:2])
                ps_s = ps_pool.tile([1, 1], F32, name="ps_s")
                nc.tensor.matmul(ps_s[:], ones_k, absw[:, 0:1], start=True,
                                 stop=True)
                nc.vector.tensor_scalar(ws1[:, 0:1], ps_s[:], 1.0 / FAN, EPS,
                                        op0=ALU.mult, op1=ALU.max)
                nc.vector.reciprocal(ws1[:, 1:2], ws1[:, 0:1])
                ps_b = ps_pool.tile([128, 2], F32, name="ps_b")
                nc.tensor.matmul(ps_b[:], ones_m, ws1[:, :], start=True, stop=True)
                wsb_cp = nc.scalar.copy(wsb, ps_b[:])

                # ---- the amax collective: one [128,2] AllReduce (a split
                # per-ct pair measured ~25us SLOWER end-to-end on the first
                # op, erasing the early-trigger win). Input hop rides the
                # Act HW queue (fast completion), trigger from gpsimd. ----
                d0 = nc.scalar.dma_start(cc_in[:], amax2)
                # d0 parks the in-order ACT queue until the amax lands: it
                # must sit after the weight/const descriptor writes (which
                # feed the whole weight pipeline) but NOT after the ws math
                for dep in (wdma[0], wdma[1], ident_dma, bias_dma):
                    add_dep_helper(d0.ins, dep.ins,
                                   reason="cc hop after const descriptors")
                nc.gpsimd.collective_compute(
                    "AllReduce", ALU.max,
                    replica_groups=[list(range(N_CORES))],
                    ins=[cc_in.opt()], outs=[cc_out.opt()],
                )
                r0 = nc.scalar.dma_start(amax2, cc_out[:])
                r1 = r0

                # ---- ternary quantize qw = clip(round(w/ws), -1, 1) on
                # vector, ordered AFTER the amax-critical reduces (it's
                # local data, needed no earlier than CC0's return), then
                # PE-transpose each 128x128 block ----
                for ot in range(2):
                    wt = wt1[ot]
                    q1 = nc.vector.tensor_scalar(wt[:], wt[:], wsb[:, 1:2], MAGIC,
                                                 op0=ALU.mult, op1=ALU.add)
                    add_dep_helper(q1.ins, am1.ins,
                                   reason="ternarize after the full amax")
                    nc.vector.tensor_scalar_sub(wt[:], wt[:], MAGIC)
                    qwb = wp.tile([128, CIN * 9], BF16, name="qwb", tag="qwb",
                                  bufs=2)
                    nc.vector.tensor_scalar(qwb[:], wt[:], -1.0, 1.0,
                                            op0=ALU.max, op1=ALU.min)
                    wv = qwb.rearrange("p (c t) -> p t c", t=9)
                    for ct in range(2):
                        for tap in range(9):
                            idx = ct * 18 + ot * 9 + tap
                            pt = pt_pool.tile([128, 128], BF16, name="pt", tag="pt")
                            nc.tensor.transpose(
                                pt[:],
                                wv[:, tap, ct * 128:(ct + 1) * 128],
                                ident_sb[:],
                            )
                            nc.scalar.copy(lhsT[:, idx, :], pt[:])
            wt_scope.__exit__(None, None, None)

            # ---- post-CC0: scales for the ct0 half; fold s_c into lhsT ----
            x0 = nc.vector.tensor_scalar_max(amax2[:, 0:1], amax2[:, 0:1], EPS)
            # anti-hoist: the in-order vector queue must finish the amax
            # reduces before parking on the CC0-gated ops
            add_dep_helper(x0.ins, am1.ins, reason="post-cc0 after amax reduces")
            nc.vector.reciprocal(xs[:, 0:1], amax2[:, 0:1])
            nc.vector.tensor_scalar_mul(xs[:, 0:1], xs[:, 0:1], 127.0)
            nc.vector.tensor_scalar(sc[:, 0:1], amax2[:, 0:1], wsb[:, 0:1],
                                    1.0 / 127.0, op0=ALU.mult, op1=ALU.mult)

            def scale_lhsT(ct, ot):
                nc.vector.tensor_scalar_mul(
                    lhsT[:, ct * 18 + ot * 9:ct * 18 + (ot + 1) * 9, :],
                    lhsT[:, ct * 18 + ot * 9:ct * 18 + (ot + 1) * 9, :],
                    sc[:, ct:ct + 1],
                )

            def quantize(n, ct, quarters=False):
                # qx = round(x*xs): vector does x*xs+MAGIC in place (fp32),
                # ACT writes qx = t - MAGIC (exact, integer-valued bf16)
                t = xres[(n, ct)]
                tv = t.rearrange("p (h w) -> p h w", w=W)
                qxa = qx[n * 2 + ct][:, PW + 1:PW + 1 + H * PW].rearrange(
                    "p (h w) -> p h w", w=PW)[:, :, 0:W]
                nh = 4 if quarters else 1
                rh = H // nh
                vop = aop = None
                for hh in range(nh):
                    rs = slice(hh * rh, (hh + 1) * rh)
                    vop = nc.vector.tensor_scalar(
                        tv[:, rs, :], tv[:, rs, :],
                        xs[:, ct:ct + 1], MAGIC,
                        op0=ALU.mult, op1=ALU.add)
                    aop = nc.scalar.activation(
                        qxa[:, rs, :], tv[:, rs, :],
                        AF.Identity, bias=negmagic)
                return vop, aop

            scale_lhsT(0, 0)
            q0v, q0a = quantize(0, 0, quarters=True)
            scale_lhsT(0, 1)
            for n in range(1, NPC):
                quantize(n, 0)

            # ---- post-CC1 scales (vector parks here while the ct0-ahead
            # conv groups below keep the PE busy) ----
            x1 = nc.vector.tensor_scalar_max(amax2[:, 1:2], amax2[:, 1:2], EPS)
            add_dep_helper(x1.ins, q0v.ins,
                           reason="post-cc1 after gating ct0 quantize")
            nc.vector.reciprocal(xs[:, 1:2], amax2[:, 1:2])
            nc.vector.tensor_scalar_mul(xs[:, 1:2], xs[:, 1:2], 127.0)
            nc.vector.tensor_scalar(sc[:, 1:2], amax2[:, 1:2], wsb[:, 0:1],
                                    1.0 / 127.0, op0=ALU.mult, op1=ALU.mult)
            scale_lhsT(1, 0)
            quantize(0, 1, quarters=True)
            scale_lhsT(1, 1)

            # ---- conv: 56 (image, cout-tile, chunk) groups, one PSUM bank
            # each. A group opens with its 9 ct0 taps and closes with the 9
            # ct1 taps + copy-out. AHEAD groups open on ct0 alone, so the
            # PE streams from CC0-land while CC1 is still in flight. ----
            def group_of(k):
                nk, rem = divmod(k, 2 * NCHUNK)
                ot, c8 = divmod(rem, NCHUNK)
                return nk, ot, c8

            def conv_half(ps, k, ct, start):
                nk, ot, c8 = group_of(k)
                base = c8 * CHUNK
                for tap in range(9):
                    di, dj = tap // 3, tap % 3
                    off = base + di * PW + dj
                    nc.tensor.matmul(
                        ps[:, 0:CHUNK],
                        lhsT[:, ct * 18 + ot * 9 + tap, :],
                        qx[nk * 2 + ct][:, off:off + CHUNK],
                        start=(start and tap == 0),
                        stop=(not start and tap == 8),
                        skip_group_check=True,
                    )

            with tc.tile_pool(name="psum_c", bufs=8, space="PSUM") as pc_pool, \
                 tc.tile_pool(name="outp", bufs=6) as op_pool:
                open_ps = {}
                for j in range(AHEAD):
                    open_ps[j] = pc_pool.tile([128, 512], F32, name="ps", tag="ps")
                    conv_half(open_ps[j], j, 0, True)
                for k in range(NGRP):
                    if k % (2 * NCHUNK) == 0 and k // (2 * NCHUNK) + 1 < NPC:
                        quantize(k // (2 * NCHUNK) + 1, 1)
                    ps = open_ps.pop(k)
                    conv_half(ps, k, 1, False)
                    nk, ot, c8 = group_of(k)
                    ob = op_pool.tile([128, OUT_CHUNK], F32, name="ob", tag="ob")
                    nc.scalar.activation(
                        ob.rearrange("p (h w) -> p h w", w=W),
                        ps[:, 0:CHUNK].rearrange(
                            "p (h w) -> p h w", w=PW)[:, :, 0:W],
                        AF.Identity, bias=bias_sb[:, ot:ot + 1])
                    nc.sync.dma_start(
                        o_flat[nk, ot * 128:(ot + 1) * 128,
                               c8 * OUT_CHUNK:(c8 + 1) * OUT_CHUNK],
                        ob[:],
                    )
                    j = k + AHEAD
                    if j < NGRP:
                        open_ps[j] = pc_pool.tile([128, 512], F32,
                                                  name="ps", tag="ps")
                        conv_half(open_ps[j], j, 0, True)

    nc.compile()
    return nc


_NC_CACHE = None


def _get_program():
    global _NC_CACHE
    if _NC_CACHE is None:
        _NC_CACHE = _build_program()
    return _NC_CACHE


def _install_ntff_hook():
    """Register the axon NTFF profiling hook (the antenv stub lacks it)."""
    try:
        import antenv
        if getattr(antenv, "axon_hooks", None) is not None:
            return
        mod = types.ModuleType("antenv.axon_hooks")
        mod._hook = None
        def set_axon_ntff_profile_hook(h):
            mod._hook = h
        def get_axon_ntff_profile_hook():
            return mod._hook
        mod.set_axon_ntff_profile_hook = set_axon_ntff_profile_hook
        mod.get_axon_ntff_profile_hook = get_axon_ntff_profile_hook
        sys.modules["antenv.axon_hooks"] = mod
        antenv.axon_hooks = mod
        from trn_agent_boot.trn_boot import _ntff_profile_via_ctypes
        set_axon_ntff_profile_hook(_ntff_profile_via_ctypes("/opt/axon/libaxon_pjrt.so"))
    except Exception:
        pass


def run(x, weight, bias, trace=False, trace_cores=None):
    x = np.ascontiguousarray(np.asarray(x, dtype=np.float32))
    weight = np.ascontiguousarray(np.asarray(weight, dtype=np.float32))
    bias = np.ascontiguousarray(np.asarray(bias, dtype=np.float32))
    assert x.shape == (N, CIN, H, W), x.shape
    nc = _get_program()
    in_maps = [
        {"x": x[c * NPC:(c + 1) * NPC], "weight": weight, "bias": bias}
        for c in range(N_CORES)
    ]
    if trace:
        _install_ntff_hook()
    res = run_bass_kernel_spmd(nc, in_maps, list(range(N_CORES)), trace=trace,
                               trace_cores=trace_cores)
    out = np.concatenate([res.results[c]["out"] for c in range(N_CORES)], axis=0)
    return out, res


def kernel(x, weight, bias):
    out, _ = run(x, weight, bias, trace=False)
    return out
